# revision 30
# baseline (speedup 1.0000x reference)
"""HGT (heterogeneous graph transformer) Bass kernel for Trainium2, 8 NeuronCores.

Strategy (dst-sharded edges):
  - Destination nodes (per type) are sharded over the 8 cores; softmax +
    segment-sum are core-local (no all-reduce).
  - Node projections are node-sharded; (k_r|v_r) tables are fp8(e4m3) and
    AllGathered in ONE shot per (layer, relation) into a pair-Shared HBM
    tensor (cores 2k/2k+1 share one physical copy -> ~2x less AG traffic,
    and NRT's shared-output collective fast path).
  - Per-edge k_r/v_r rows come from one indirect DMA per 128-edge tile
    (fp8 rows, 256B). The ~1us SWDGE descgen per instruction on the Pool
    engine is the kernel's throughput cap (~1.4us/tile); multi-column
    offset APs and InstDMAGatherAnt both fail on this HW/ucode, so
    per-tile gathers are the only correct primitive.
  - q[dst] is expanded on the TensorEngine with a one-hot matmul (edges
    sorted by dst => q rows of a window are SBUF-local); segment
    softmax/sum are one-hot matmuls accumulating in PSUM per 128-dst
    window; max-subtraction is skipped (logits are O(0.1)).
  - a_rel/m_rel/p_rel/scale folded into effective weights on the host.
  - ALL dense work (out-projection chunks, next-layer node-pass chunks,
    AllGather dispatches) is spliced into the edge phases as
    dependency-gated hooks fired between windows, so no engine queue
    drains behind a dense block and the Pool gather stream never stops
    (in-order queues head-of-line-block otherwise).
  - h tables / dense weights are bf16 (halves dense DMA, 2x PE rate);
    kv is fp8 end-to-end (DVE reads fp8 operands directly).
"""
import os
import sys

import numpy as np

try:
    import concourse  # noqa: F401
except ImportError:  # pragma: no cover
    sys.path.insert(0, "/opt/trn_rl_repo")

import ml_dtypes

import concourse.bacc as bacc
import concourse.bass as bass
import concourse.tile as tile
from concourse import mybir
from concourse.bass_utils import run_bass_kernel_spmd

f32 = mybir.dt.float32
bf16 = mybir.dt.bfloat16
fp8 = mybir.dt.float8e4
i32 = mybir.dt.int32
AF = mybir.ActivationFunctionType
ALU = mybir.AluOpType
BF = ml_dtypes.bfloat16

FULL_CFG = dict(N=100000, E=500000, HID=128, H=4, D=32, L=2, NC=8)
GMAX = 6   # max edge tiles per PSUM compute group (PSUM: 8 banks budget)
MAXM = 7   # max one-hot masks per compute group (7*256B < 1 PSUM bank)
NCH = 7    # AllGather chunks per kv table (NP must divide by NCH*128)


def _blockdiag(a):  # a: [H, D, D] -> [H*D, H*D]
    H, D, _ = a.shape
    out = np.zeros((H * D, H * D), np.float32)
    for h in range(H):
        out[h * D:(h + 1) * D, h * D:(h + 1) * D] = a[h]
    return out


def host_prep(inputs, cfg):
    N, E, HID, H, D, L, NC = (cfg[k] for k in ("N", "E", "HID", "H", "D", "L", "NC"))
    NSH = N // NC
    W = (NSH + 127) // 128
    NP = W * 128

    ip = {k: np.asarray(v) for k, v in inputs.items()}
    rel_st = [0, 1]
    rel_dt = [1, 0]
    edges = [ip["edge_ui"], ip["edge_iu"]]
    nch = NCH if NP % (NCH * 128) == 0 else (2 if NP % 256 == 0 else 1)
    CHR = NP // nch  # rows per AllGather chunk (chunk-major kvfull layout)

    # ---- effective weights ----
    scale = 1.0 / np.sqrt(D)
    W3 = np.zeros((L, 2, HID, 3 * HID), np.float32)  # (l, r): [Wk_eff|Wv_eff|Wq_eff(t=r)]
    b3 = np.zeros((L, 2, 3 * HID), np.float32)
    for l in range(L):
        for r in range(2):
            st, dt = rel_st[r], rel_dt[r]
            BDa = _blockdiag(ip["a_rel"][l, r])
            BDm = _blockdiag(ip["m_rel"][l, r])
            W3[l, r, :, 0:HID] = ip["Wk"][l, st] @ BDa
            b3[l, r, 0:HID] = ip["bk"][l, st] @ BDa
            W3[l, r, :, HID:2 * HID] = ip["Wv"][l, st] @ BDm
            b3[l, r, HID:2 * HID] = ip["bv"][l, st] @ BDm
        for t in range(2):
            r_of = 1 - t  # relation whose dst type is t
            pscale = np.repeat(ip["p_rel"][l, r_of] * scale, D)
            W3[l, t, :, 2 * HID:3 * HID] = ip["Wq"][l, t] * pscale[None, :]
            b3[l, t, 2 * HID:3 * HID] = ip["bq"][l, t] * pscale
    beta = 1.0 / (1.0 + np.exp(-ip["skip"]))  # [L, T]

    # ---- edge schedules (identical across cores) ----
    # Windows are packed into groups of up to KGRP consecutive windows that
    # share tiles (dense packing; boundary tiles carry one one-hot mask per
    # window they touch). Pool-engine gather count ~= tiles, so packing cuts
    # the dominant per-tile SWDGE cost by ~11%.
    KGRP = W   # one group per relation: dense tile packing (true minimum)
    grpK = [W]
    win2g = []
    for g, k in enumerate(grpK):
        win2g += [g] * k

    def prep_rel(e):
        src, dst = e[0].astype(np.int64), e[1].astype(np.int64)
        i_loc = src % NSH
        gsrc = (src // NSH) * NP + i_loc  # core-major (single-shot AllGather)
        per_core = []
        counts = np.zeros((NC, W), np.int64)
        for c in range(NC):
            sel = (dst // NSH) == c
            s_c = gsrc[sel]
            dl_c = dst[sel] - c * NSH
            order = np.argsort(dl_c, kind="stable")
            s_c, dl_c = s_c[order], dl_c[order]
            counts[c] = np.bincount(dl_c // 128, minlength=W)
            per_core.append((s_c, dl_c))
        wstart = np.concatenate([[0], np.cumsum(counts, axis=1)[:, :-1].T]).T \
            if False else np.concatenate(
                [np.zeros((NC, 1), np.int64), np.cumsum(counts, axis=1)[:, :-1]],
                axis=1)
        groups = []   # per group: dict(nt, tmasks=[(ti,wi)...], wb={wi:(lo,hi)})
        NT = 0
        wlo = 0
        while wlo < W:
            # grow the group while the 2-deep accumulator-ring invariant
            # holds (window wi may only share tiles with its neighbor)
            k = 0
            wb = {}
            while wlo + k < W:
                w = wlo + k
                los, his = [], []
                for c in range(NC):
                    pre = int(counts[c, wlo:w].sum())
                    n = int(counts[c, w])
                    if n:
                        los.append(pre // 128)
                        his.append((pre + n - 1) // 128)
                lo = min(los) if los else (wb[k - 1][1] if k else 0)
                hi = max(his) if his else lo
                if k >= 2 and lo <= wb[k - 2][1]:
                    break  # would need a 3rd live accumulator
                wb[k] = (lo, hi)
                k += 1
            cnt_g = counts[:, wlo:wlo + k].sum(axis=1)
            nt = max(1, int((int(cnt_g.max()) + 127) // 128))
            tmasks = []
            for t in range(nt):
                for wi in range(k):
                    if wb[wi][0] <= min(t, nt - 1) and t <= wb[wi][1]:
                        tmasks.append((t, wi))
            groups.append(dict(k=k, w0=wlo, nt=nt, tmasks=tmasks, wb=wb,
                               tbase=NT))
            NT += nt
            wlo += k
        NM = sum(len(g["tmasks"]) for g in groups)
        # per-core data: gather idx per tile, dloc per (tile, window-mask)
        idx_src = np.zeros((NC, NT * 128), np.int32)
        dloc = np.full((NC, NM * 128), 128.0, np.float32)  # 128 => miss
        for c in range(NC):
            s_c, dl_c = per_core[c]
            for g in groups:
                wlo = g["w0"]
                a0 = int(wstart[c, wlo])
                n_g = int(counts[c, wlo:wlo + g["k"]].sum())
                sl0 = g["tbase"] * 128
                idx_src[c, sl0:sl0 + n_g] = s_c[a0:a0 + n_g]
            mcol = 0
            for g in groups:
                wlo = g["w0"]
                for (t, wi) in g["tmasks"]:
                    w = wlo + wi
                    a = int(wstart[c, w])
                    n = int(counts[c, w])
                    # slots of window w on this core within the group stream
                    s_in_g = int(wstart[c, w] - wstart[c, wlo])
                    lo_s = max(s_in_g, t * 128)
                    hi_s = min(s_in_g + n, (t + 1) * 128)
                    if hi_s > lo_s:
                        aa = a + (lo_s - s_in_g)
                        dloc[c, mcol * 128 + (lo_s - t * 128):
                             mcol * 128 + (hi_s - t * 128)] = \
                            (dl_c[aa:aa + (hi_s - lo_s)] % 128).astype(np.float32)
                    mcol += 1
        return groups, NT, NM, idx_src, dloc

    schedules = []
    meta = []
    for r in range(2):
        groups, NT, NM, idx_src, dloc = prep_rel(edges[r])
        schedules.append((groups, NT, NM))
        meta.append((idx_src, dloc))

    # ---- per-core input arrays ----
    NTtot = schedules[0][1] + schedules[1][1]
    NMtot = schedules[0][2] + schedules[1][2]
    xs = [ip["x_user"].astype(np.float32), ip["x_item"].astype(np.float32)]
    in_maps = []
    for c in range(NC):
        x_fm = np.zeros((2, HID, NP), BF)
        for t in range(2):
            x_fm[t, :, :NSH] = xs[t][c * NSH:(c + 1) * NSH].T.astype(BF)
        idx_cat = np.concatenate(
            [meta[0][0][c], meta[1][0][c]]).reshape(NTtot, 128).T
        dl = np.concatenate([meta[0][1][c], meta[1][1][c]])
        dloc_col = dl.reshape(NMtot, 128).T.astype(np.float32).copy()
        in_maps.append({
            "x_fm": x_fm,
            "idx_src": np.ascontiguousarray(idx_cat.astype(np.int32)),
            "dloc_col": np.ascontiguousarray(dloc_col),
        })

    # bias cols [128, NB] f32: 0,1 = b_in; 2.. = beta*bo (l,t)
    b_list = [ip["b_in"][0], ip["b_in"][1]]
    for l in range(L):
        for t in range(2):
            b_list.append(beta[l, t] * ip["bo"][l, t])
    Bcols = np.stack(b_list).astype(np.float32)

    bias_nz = [[bool(np.any(b3[l, r] != 0)) for r in range(2)] for l in range(L)]
    iota = np.tile(np.arange(128, dtype=np.float32), (128, 1))
    consts = {
        "bias_nz": bias_nz,
        "iota_row": np.tile(iota, (1, MAXM)).astype(BF),  # [128, MAXM*128]
        "ident": np.eye(128, dtype=np.float32).astype(BF),
        "W3": W3.reshape(L * 2, HID, 3 * HID).astype(BF),
        "Win": ip["W_in"].astype(np.float32).astype(BF),
        "Wo_bf": ip["Wo"].astype(np.float32).reshape(L * 2, HID, HID).astype(BF),
        "b3": b3.reshape(1, L * 2 * 3 * HID).astype(np.float32),
        "Bcols": Bcols,
        "ones1f": np.ones((1, 128), np.float32),
    }
    dims = dict(NSH=NSH, W=W, NP=NP, NTtot=NTtot, NMtot=NMtot,
                NCHR=nch, win2g=win2g, NG=len(grpK))
    return in_maps, consts, {}, schedules, dims, beta


def build_program(cfg, consts, bases, schedules, dims, beta, sim_gelu=False):
    N, E, HID, H, D, L, NC = (cfg[k] for k in ("N", "E", "HID", "H", "D", "L", "NC"))
    NSH, W, NP, NTtot = dims["NSH"], dims["W"], dims["NP"], dims["NTtot"]
    NMtot, win2g, NG = dims["NMtot"], dims["win2g"], dims["NG"]
    NPALL = NP * NC
    rel_dt = [1, 0]
    NB = consts["Bcols"].shape[0]
    CHD = 448 if NP % 448 == 0 else 128      # dense (psum) chunk width
    nch = dims["NCHR"]
    CHN = NP // nch                           # node h chunk == AG chunk rows
    assert NP % CHD == 0 and NP % CHN == 0 and CHN % 128 == 0

    nc = bacc.Bacc("TRN2", target_bir_lowering=False, debug=False, num_devices=NC)

    x_fm = nc.dram_tensor("x_fm", [2, HID, NP], bf16, kind="ExternalInput")
    idx_src = nc.dram_tensor("idx_src", [128, NTtot], i32, kind="ExternalInput")
    dloc_col_d = nc.dram_tensor("dloc_col", [128, NMtot], f32, kind="ExternalInput")
    it_row_d = nc.dram_tensor("iota_row", [128, MAXM * 128], bf16, kind="ExternalInput")
    ident_d = nc.dram_tensor("ident", [128, 128], bf16, kind="ExternalInput")
    W3_d = nc.dram_tensor("W3", [L * 2, HID, 3 * HID], bf16, kind="ExternalInput")
    Win_d = nc.dram_tensor("Win", [2, HID, HID], bf16, kind="ExternalInput")
    Wo_d = nc.dram_tensor("Wo_bf", [L * 2, HID, HID], bf16, kind="ExternalInput")
    b3_d = nc.dram_tensor("b3", [1, L * 2 * 3 * HID], f32, kind="ExternalInput")
    Bcols_d = nc.dram_tensor("Bcols", [NB, HID], f32, kind="ExternalInput")
    ones1f_d = nc.dram_tensor("ones1f", [1, 128], f32, kind="ExternalInput")
    out_d = nc.dram_tensor("out", [2, HID, NP], f32, kind="ExternalOutput")

    with tile.TileContext(nc) as tc:
        with tc.tile_pool(name="persist", bufs=1) as pp, \
             tc.tile_pool(name="dram", bufs=1, space="DRAM") as dp, \
             tc.tile_pool(name="wk_sb", bufs=3) as sb3, \
             tc.tile_pool(name="wk_sb2", bufs=2) as sb2, \
             tc.tile_pool(name="gath", bufs=8) as gpool, \
             tc.tile_pool(name="edge8", bufs=2) as sb8, \
             tc.tile_pool(name="ps_edge", bufs=2, space="PSUM") as ps_e, \
             tc.tile_pool(name="ps_dense", bufs=2, space="PSUM") as ps_d:

            # --- persistent SBUF ---
            it_row = pp.tile([128, MAXM, 128], bf16)
            nc.sync.dma_start(it_row[:], it_row_d[:].rearrange(
                "p (g d) -> p g d", g=MAXM))
            ident = pp.tile([128, 128], bf16)
            nc.sync.dma_start(ident[:], ident_d[:])
            onesf = pp.tile([1, 128], f32)
            nc.sync.dma_start(onesf[:], ones1f_d[:])
            idxs = pp.tile([128, NTtot], i32)
            nc.sync.dma_start(idxs[:], idx_src[:])
            dloc_col = pp.tile([128, NMtot], f32)
            nc.sync.dma_start(dloc_col[:], dloc_col_d[:])
            w3sb = pp.tile([128, L * 2, 3 * HID], bf16)
            nc.sync.dma_start(w3sb[:], W3_d[:].rearrange("k p d -> p k d"))
            winsb = pp.tile([128, 2, HID], bf16)
            nc.sync.dma_start(winsb[:], Win_d[:].rearrange("k p d -> p k d"))
            wosb = pp.tile([128, L * 2, HID], bf16)
            nc.sync.dma_start(wosb[:], Wo_d[:].rearrange("k p d -> p k d"))
            b3sb = pp.tile([1, L * 2 * 3 * HID], f32)
            nc.sync.dma_start(b3sb[:], b3_d[:])
            bcols = pp.tile([128, NB], f32)
            nc.sync.dma_start(bcols[:], Bcols_d[:].rearrange("k d -> d k"))

            q_sb = [pp.tile([128, W, 128], bf16, name=f"q_sb{t}") for t in range(2)]
            g_fm = [pp.tile([128, NP], bf16, name=f"g_fm{t}") for t in range(2)]

            hA = [dp.tile([128, NP], bf16, name=f"hA{t}") for t in range(2)]
            hB = [dp.tile([128, NP], bf16, name=f"hB{t}") for t in range(2)]
            kvloc = [dp.tile([NP, 256], fp8, name=f"kvloc{r}") for r in range(2)]
            kvfull = [[dp.tile([NPALL, 256], fp8, name=f"kvfull{l}{r}",
                        addr_space="Shared")
                       for r in range(2)] for l in range(L)]
            rg = [list(range(NC))]

            def b3row(l, r, lo, hi):  # bias row slice [1, hi-lo]
                base = (l * 2 + r) * 3 * HID
                return b3sb[:, base + lo:base + hi]

            def ag_full(l, r):
                # One-shot AllGather into the pair-Shared kvfull (Shared
                # outputs allow a single writer instruction only). Core c's
                # kvloc lands at rows [c*NP, (c+1)*NP) (core-major gsrc).
                nc.gpsimd.collective_compute(
                    "AllGather", ALU.bypass, replica_groups=rg,
                    ins=[kvloc[r][:, :]],
                    outs=[kvfull[l][r][:, :]])

            # dense projection pass over the node shard, writing kv and/or q.
            def node_pass(l, r, h_src, do_kv, do_q):
                lo = 0 if do_kv else 2 * HID
                hi = 3 * HID if do_q else 2 * HID
                ncols = hi - lo
                for jc in range(NP // CHN):
                    hch = sb3.tile([128, CHN], bf16, tag="hch")
                    nc.sync.dma_start(hch[:], h_src[:, jc * CHN:(jc + 1) * CHN])
                    for k in range(CHN // 128):
                        w = jc * (CHN // 128) + k
                        ps = ps_d.tile([128, 3 * HID], f32, tag="dense")
                        bias_nz = consts["bias_nz"][l][r]
                        nc.tensor.matmul(
                            out=ps[:, :ncols], lhsT=hch[:, k * 128:(k + 1) * 128],
                            rhs=w3sb[:, l * 2 + r, lo:hi], start=True,
                            stop=not bias_nz)
                        if bias_nz:
                            nc.tensor.matmul(
                                out=ps[:, :ncols], lhsT=onesf[:],
                                rhs=b3row(l, r, lo, hi), start=False, stop=True)
                        if do_kv:
                            kvt = sb3.tile([128, 256], fp8, tag="kvt")
                            nc.scalar.activation(kvt[:], ps[:, 0:256], AF.Copy)
                            nc.sync.dma_start(
                                kvloc[r][w * 128:(w + 1) * 128, :], kvt[:])
                        if do_q:
                            nc.vector.tensor_copy(
                                q_sb[r][:, w, :], ps[:, ncols - HID:ncols])


            def flush_window(dt, w, pw):
                # normalize window agg, transpose into g_fm
                zrw = sb8.tile([128, 4], f32, tag="zrw")
                nc.vector.tensor_scalar(out=zrw[:], in0=pw[:, 128:132],
                                        scalar1=1e-16, scalar2=None,
                                        op0=ALU.add)
                nc.vector.reciprocal(zrw[:], zrw[:])
                gt = sb8.tile([128, 128], bf16, tag="gt")
                nc.vector.tensor_tensor(
                    out=gt[:].rearrange("p (h d) -> p h d", h=H),
                    in0=pw[:, 0:128].rearrange("p (h d) -> p h d", h=H),
                    in1=zrw[:].to_broadcast([128, H, D]),
                    op=ALU.mult)
                psgt = ps_e.tile([128, 128], bf16, tag="st")
                nc.tensor.transpose(out=psgt[:], in_=gt[:], identity=ident[:])
                nc.scalar.activation(g_fm[dt][:, w * 128:(w + 1) * 128],
                                     psgt[:], AF.Copy)

            def edge_phase(l, r, tbase, mbase, hooks=None):
                # hooks: ordered [(min_group, closure)] spliced between
                # window-groups, fired strictly in list order (at most 4 per
                # group) once min_group's flush has been emitted. min_group
                # must be >= the group whose flush produces the closure's
                # input (dependency-order within the list).
                pending = list(hooks or [])
                wdone = 0
                groups, NT, NM = schedules[r]
                dt = rel_dt[r]
                for gidx, gd in enumerate(groups):
                    K, w0, nt, tmasks, wb = (gd["k"], gd["w0"], gd["nt"],
                                             gd["tmasks"], gd["wb"])
                    tb0 = tbase + gd["tbase"]
                    # per-window accumulators (zero-region = one full bank:
                    # interleaved groups must live in separate banks; only
                    # adjacent windows overlap, ring of 2 suffices)
                    pswin = {}
                    flushed = [False] * K
                    # split group tiles into compute sub-groups bounded by
                    # GMAX tiles and MAXM one-hot masks
                    nmask_of = [0] * nt
                    for (t, wi) in tmasks:
                        nmask_of[t] += 1
                    ta = 0
                    moff = 0  # mask offset within group
                    while ta < nt:
                        tb_ = ta
                        nm = 0
                        while (tb_ < nt and tb_ - ta < GMAX
                               and nm + nmask_of[tb_] <= MAXM):
                            nm += nmask_of[tb_]
                            tb_ += 1
                        Gt = tb_ - ta
                        sg_masks = [mk for mk in tmasks if ta <= mk[0] < tb_]
                        NMsg = len(sg_masks)
                        mb = mbase + moff
                        ts = tb0 + ta
                        # gather the sub-group's kv rows (one DMA per
                        # 128-edge tile; batched offsets mislower on HW)
                        kvg = gpool.tile([128, GMAX, 256], fp8, tag="g")
                        for i in range(Gt):
                            tt = ts + i
                            nc.gpsimd.indirect_dma_start(
                                out=kvg[:, i, :], out_offset=None,
                                in_=kvfull[l][r][:],
                                in_offset=bass.IndirectOffsetOnAxis(
                                    ap=idxs[:, tt:tt + 1], axis=0))
                        # one-hot masks for every (tile, window) pair
                        S2 = sb8.tile([128, MAXM, 128], bf16, tag="S")
                        nc.vector.tensor_tensor(
                            out=S2[:, :NMsg, :], in0=it_row[:, :NMsg, :],
                            in1=dloc_col[:, mb:mb + NMsg].to_broadcast(
                                [128, NMsg, 128]),
                            op=ALU.is_equal)
                        psst = ps_e.tile([128, MAXM, 128], bf16, tag="st")
                        for m in range(NMsg):
                            nc.tensor.transpose(out=psst[:, m, :],
                                                in_=S2[:, m, :],
                                                identity=ident[:])
                        St2 = sb8.tile([128, MAXM, 128], bf16, tag="St")
                        nc.scalar.activation(St2[:, :NMsg, :], psst[:, :NMsg, :],
                                             AF.Copy)
                        # q[dst] per slot: accumulate this tile's masks
                        psqe = ps_e.tile([128, GMAX, 128], f32, tag="qe",
                                         bufs=1)
                        for i in range(Gt):
                            mks = [m for m, mk in enumerate(sg_masks)
                                   if mk[0] == ta + i]
                            for j, m in enumerate(mks):
                                nc.tensor.matmul(
                                    out=psqe[:, i, :], lhsT=St2[:, m, :],
                                    rhs=q_sb[dt][:, w0 + sg_masks[m][1], :],
                                    start=(j == 0), stop=(j == len(mks) - 1))
                        qk = sb8.tile([128, GMAX, 128], bf16, tag="qk")
                        nc.vector.tensor_tensor(
                            out=qk[:, :Gt, :], in0=psqe[:, :Gt, :],
                            in1=kvg[:, 0:Gt, 0:128], op=ALU.mult)
                        lg = sb8.tile([128, GMAX, 4], f32, tag="lg")
                        nc.vector.tensor_reduce(
                            out=lg[:, :Gt, :],
                            in_=qk[:, :Gt, :].rearrange(
                                "p g (h d) -> p (g h) d", h=H),
                            axis=mybir.AxisListType.X, op=ALU.add)
                        pay = sb8.tile([128, GMAX, 132], bf16, tag="pay")
                        nc.scalar.activation(pay[:, :Gt, 128:132], lg[:, :Gt, :],
                                             AF.Exp)
                        nc.vector.tensor_tensor(
                            out=pay[:, :Gt, 0:128].rearrange(
                                "p g (h d) -> p g h d", h=H),
                            in0=kvg[:, 0:Gt, 128:256].rearrange(
                                "p g (h d) -> p g h d", h=H),
                            in1=pay[:, :Gt, 128:132].to_broadcast(
                                [128, Gt, H, D]),
                            op=ALU.mult)
                        # per-window segment accumulation (one matmul per mask)
                        for m, (t, wi) in enumerate(sg_masks):
                            if wi not in pswin:
                                pswin[wi] = ps_e.tile([128, 132], f32,
                                                      tag="win", name="pswin")
                            nc.tensor.matmul(
                                out=pswin[wi][:], lhsT=S2[:, m, :],
                                rhs=pay[:, t - ta, :],
                                start=(t == wb[wi][0]), stop=(t == wb[wi][1]))
                        ta = tb_
                        moff += NMsg
                        # flush windows whose last tile has completed
                        for wi in range(K):
                            if not flushed[wi] and wb[wi][1] < ta:
                                flush_window(dt, w0 + wi, pswin.pop(wi))
                                flushed[wi] = True
                                wdone += 1
                        fired = 0
                        while pending and pending[0][0] < wdone and fired < 2:
                            pending.pop(0)[1]()
                            fired += 1
                    mbase += len(tmasks)
                    for wi in range(K):
                        if not flushed[wi]:
                            flush_window(dt, w0 + wi, pswin.pop(wi))
                            flushed[wi] = True
                            wdone += 1
                for _, fn in pending:
                    fn()

            def bulk_gelu(t, lo, hi):
                if not sim_gelu:
                    nc.scalar.activation(g_fm[t][:, lo:hi], g_fm[t][:, lo:hi],
                                         AF.Gelu)
                else:
                    tmp = sb2.tile([128, NP], f32, tag="sgl")
                    g = g_fm[t][:, lo:hi]
                    tm = tmp[:, lo:hi]
                    nc.vector.tensor_tensor(out=tm, in0=g, in1=g, op=ALU.mult)
                    nc.vector.tensor_scalar(out=tm, in0=tm, scalar1=0.044715,
                                            scalar2=1.0, op0=ALU.mult, op1=ALU.add)
                    nc.vector.tensor_tensor(out=tm, in0=tm, in1=g, op=ALU.mult)
                    nc.scalar.activation(tm, tm, AF.Tanh, scale=0.7978845608028654)
                    nc.vector.tensor_scalar(out=tm, in0=tm, scalar1=1.0, scalar2=0.5,
                                            op0=ALU.add, op1=ALU.mult)
                    nc.vector.tensor_tensor(out=g, in0=tm, in1=g, op=ALU.mult)

            def out_chunk(l, t, h_src, dst, last, j):
                bb = 2 + l * 2 + t
                coef = float((1.0 - beta[l, t]) + (1.0 if l > 0 else 0.0))
                sl = slice(j * CHD, (j + 1) * CHD)
                bulk_gelu(t, j * CHD, (j + 1) * CHD)
                ps = ps_d.tile([128, CHD], f32, tag="dense")
                nc.tensor.matmul(out=ps[:], lhsT=wosb[:, l * 2 + t, :],
                                 rhs=g_fm[t][:, sl], start=True, stop=True)
                a1 = sb2.tile([128, CHD], f32, tag="a1")
                nc.vector.tensor_scalar(
                    out=a1[:], in0=ps[:], scalar1=float(beta[l, t]),
                    scalar2=bcols[:, bb:bb + 1], op0=ALU.mult, op1=ALU.add)
                hch = sb2.tile([128, CHD], bf16, tag="hcho")
                nc.sync.dma_start(hch[:], h_src[:, sl])
                hn = sb2.tile([128, CHD], f32 if last else bf16, tag="hn")
                nc.vector.scalar_tensor_tensor(
                    out=hn[:], in0=hch[:], scalar=coef, in1=a1[:],
                    op0=ALU.mult, op1=ALU.add)
                nc.sync.dma_start(dst[:, sl], hn[:])

            def out_chunks(l, t, h_src, dst, last):
                return [(lambda j=j: out_chunk(l, t, h_src, dst, last, j))
                        for j in range(NP // CHD)]

            def node_pass_chunk(l, r, h_src, do_kv, do_q, jc, ag=None):
                # one CHN-wide chunk of node_pass (see node_pass)
                lo = 0 if do_kv else 2 * HID
                hi = 3 * HID if do_q else 2 * HID
                ncols = hi - lo
                hch = sb3.tile([128, CHN], bf16, tag="hch")
                nc.sync.dma_start(hch[:], h_src[:, jc * CHN:(jc + 1) * CHN])
                for k in range(CHN // 128):
                    w = jc * (CHN // 128) + k
                    ps = ps_d.tile([128, 3 * HID], f32, tag="dense")
                    bias_nz = consts["bias_nz"][l][r]
                    nc.tensor.matmul(
                        out=ps[:, :ncols], lhsT=hch[:, k * 128:(k + 1) * 128],
                        rhs=w3sb[:, l * 2 + r, lo:hi], start=True,
                        stop=not bias_nz)
                    if bias_nz:
                        nc.tensor.matmul(
                            out=ps[:, :ncols], lhsT=onesf[:],
                            rhs=b3row(l, r, lo, hi), start=False, stop=True)
                    if do_kv:
                        kvt = sb3.tile([128, 256], fp8, tag="kvt")
                        nc.scalar.activation(kvt[:], ps[:, 0:256], AF.Copy)
                        nc.sync.dma_start(
                            kvloc[r][w * 128:(w + 1) * 128, :], kvt[:])
                    if do_q:
                        nc.vector.tensor_copy(
                            q_sb[r][:, w, :], ps[:, ncols - HID:ncols])
                if ag is not None and jc == NP // CHN - 1:
                    ag_full(ag[0], ag[1])

            def node_pass_chunks(l, r, h_src, do_kv, do_q, ag=None):
                return [(lambda jc=jc: node_pass_chunk(l, r, h_src, do_kv,
                                                       do_q, jc, ag))
                        for jc in range(NP // CHN)]

            def input_proj(t, dst):
                for j in range(NP // CHD):
                    sl = slice(j * CHD, (j + 1) * CHD)
                    xt = sb2.tile([128, CHD], bf16, tag="xt")
                    nc.sync.dma_start(xt[:], x_fm[t, :, sl])
                    ps = ps_d.tile([128, CHD], f32, tag="dense")
                    nc.tensor.matmul(out=ps[:], lhsT=winsb[:, t, :], rhs=xt[:],
                                     start=True, stop=True)
                    ht = sb2.tile([128, CHD], bf16, tag="ht")
                    nc.scalar.activation(ht[:], ps[:], AF.Relu,
                                         bias=bcols[:, t:t + 1], scale=1.0)
                    nc.sync.dma_start(dst[:, sl], ht[:])

            # ---------------- schedule ----------------
            # All dense work (out-projection chunks, next-layer node-pass
            # chunks, AllGather chunks) is spliced INTO the edge phases as
            # dependency-gated hooks, so no engine queue ever drains behind
            # a dense block and the Pool gather stream never stops:
            #   edge(l, rF)  hooks: AG(l, rS) chunks  [inputs ready at start]
            #   edge(l, rS)  hooks: out(l, tF) packed early; kv node-pass +
            #                AG for (l+1, rF2); out(l, tS) gated on this
            #                phase's window flushes; q node-pass for (l+1,
            #                rS2) gated on out(l, tS) chunks.
            #   edge(l+1, rF2) hooks: kv node-pass + AG for (l+1, rS2) and
            #                q node-pass for (l+1, rF2)  [inputs ready]
            rorder = [[0, 1] if l % 2 == 0 else [1, 0] for l in range(L)]
            tb = [0, schedules[0][1]]   # tile base per relation
            mb_ = [0, schedules[0][2]]  # mask-column base per relation
            NNC = NP // CHN             # node-pass chunks

            h_cur = hA
            rF, rS = rorder[0]
            input_proj(rF, hA[rF])
            node_pass(0, rF, hA[rF], do_kv=True, do_q=False)
            ag_full(0, rF)
            input_proj(rS, hA[rS])
            node_pass(0, rS, hA[rS], do_kv=True, do_q=True)
            node_pass(0, rF, hA[rF], do_kv=False, do_q=True)
            hooksA = [(min(55, W - 2), lambda: ag_full(0, rS))]

            for l in range(L):
                rF, rS = rorder[l]
                last = l == L - 1
                l2 = l + 1
                h_nxt = hB if l == 0 else None
                edge_phase(l, rF, tb[rF], mb_[rF], hooks=hooksA)

                tF_out = rel_dt[rF]
                tS_out = rel_dt[rS]
                dstF = (h_nxt[tF_out] if not last else out_d[tF_out])
                dstS = (h_nxt[tS_out] if not last else out_d[tS_out])
                hooks2 = [(2 + j, fn) for j, fn in
                          enumerate(out_chunks(l, tF_out, h_cur[tF_out],
                                               dstF, last))]
                # out(l, tS) chunk j needs this phase's window
                # floor(((j+1)*CHD-1)/128) flushed
                hooks2 += [((CHD * (j + 1) - 1) // 128, fn) for j, fn in
                           enumerate(out_chunks(l, tS_out, h_cur[tS_out],
                                                dstS, last))]
                hooks2.sort(key=lambda e: e[0])
                edge_phase(l, rS, tb[rS], mb_[rS], hooks=hooks2)

                if not last:
                    rF2, rS2 = rorder[l2]
                    for fn in node_pass_chunks(l2, rF2, h_nxt[rF2], True,
                                               False, ag=(l2, rF2)):
                        fn()
                    for fn in node_pass_chunks(l2, rS2, h_nxt[rS2], False,
                                               True):
                        fn()

                if not last:
                    hooksA = []
                    npkv = node_pass_chunks(l2, rS2, h_nxt[rS2], True, False,
                                            ag=(l2, rS2))
                    npq = node_pass_chunks(l2, rF2, h_nxt[rF2], False, True)
                    for jc in range(NNC):
                        hooksA.append((12 + 3 * jc, npkv[jc]))
                        hooksA.append((2 + 2 * jc, npq[jc]))
                    hooksA.sort(key=lambda e: e[0])
                    h_cur = hB

    nc.finalize()
    return nc


def run(inputs, cfg=None, trace=False, trace_cores=None, sim=False):
    cfg = cfg or FULL_CFG
    NC = cfg["NC"]
    core_maps, consts, bases, schedules, dims, beta = host_prep(inputs, cfg)
    nc = build_program(cfg, consts, bases, schedules, dims, beta, sim_gelu=sim)
    in_maps = []
    for c in range(NC):
        m = dict(core_maps[c])
        for k in ("iota_row", "ident", "W3", "Win", "Wo_bf", "b3", "Bcols",
                  "ones1f"):
            m[k] = consts[k]
        in_maps.append(m)
    if sim:
        from concourse.bass_interp import MultiCoreSim

        msim = MultiCoreSim(nc, num_cores=NC, trace=False,
                            require_finite=False, require_nnan=False)
        cores = [msim.cores[c] for c in range(NC)]
        for c in range(NC):
            for name, arr in in_maps[c].items():
                cores[c].tensor(name)[:] = arr
        msim.simulate(check_with_hw=False)

        class R:
            exec_time_ns = None
            results = [{"out": np.asarray(cores[c].tensor("out"))}
                       for c in range(NC)]
        res = R()
    else:
        res = run_bass_kernel_spmd(nc, in_maps, core_ids=list(range(NC)),
                                   trace=trace, trace_cores=trace_cores)
    NSH, NP = dims["NSH"], dims["NP"]
    out = np.empty((2, cfg["N"], cfg["HID"]), np.float32)
    for c in range(NC):
        o = res.results[c]["out"]
        for t in range(2):
            out[t, c * NSH:(c + 1) * NSH] = o[t, :, :NSH].T
    return out, res


def kernel(**inputs):
    out, _ = run(inputs, FULL_CFG, trace=False)
    return out



# revision 31
# speedup vs baseline: 1.0743x; 1.0743x over previous
"""HGT (heterogeneous graph transformer) Bass kernel for Trainium2, 8 NeuronCores.

Strategy (dst-sharded edges):
  - Destination nodes (per type) are sharded over the 8 cores; softmax +
    segment-sum are core-local (no all-reduce).
  - Node projections are node-sharded; (k_r|v_r) tables are fp8(e4m3) and
    AllGathered in ONE shot per (layer, relation) into a pair-Shared HBM
    tensor (cores 2k/2k+1 share one physical copy -> ~2x less AG traffic,
    and NRT's shared-output collective fast path).
  - Per-edge k_r/v_r rows come from one indirect DMA per 128-edge tile
    (fp8 rows, 256B). The ~1us SWDGE descgen per instruction on the Pool
    engine is the kernel's throughput cap (~1.4us/tile); multi-column
    offset APs and InstDMAGatherAnt both fail on this HW/ucode, so
    per-tile gathers are the only correct primitive.
  - q[dst] is expanded on the TensorEngine with a one-hot matmul (edges
    sorted by dst => q rows of a window are SBUF-local); segment
    softmax/sum are one-hot matmuls accumulating in PSUM per 128-dst
    window; max-subtraction is skipped (logits are O(0.1)).
  - a_rel/m_rel/p_rel/scale folded into effective weights on the host.
  - ALL dense work (out-projection chunks, next-layer node-pass chunks,
    AllGather dispatches) is spliced into the edge phases as
    dependency-gated hooks fired between windows, so no engine queue
    drains behind a dense block and the Pool gather stream never stops
    (in-order queues head-of-line-block otherwise).
  - h tables / dense weights are bf16 (halves dense DMA, 2x PE rate);
    kv is fp8 end-to-end (DVE reads fp8 operands directly).
"""
import os
import sys

import numpy as np

try:
    import concourse  # noqa: F401
except ImportError:  # pragma: no cover
    sys.path.insert(0, "/opt/trn_rl_repo")

import ml_dtypes

import concourse.bacc as bacc
import concourse.bass as bass
import concourse.tile as tile
from concourse import mybir
from concourse.bass_utils import run_bass_kernel_spmd

f32 = mybir.dt.float32
bf16 = mybir.dt.bfloat16
fp8 = mybir.dt.float8e4
i32 = mybir.dt.int32
AF = mybir.ActivationFunctionType
ALU = mybir.AluOpType
BF = ml_dtypes.bfloat16

FULL_CFG = dict(N=100000, E=500000, HID=128, H=4, D=32, L=2, NC=8)
GMAX = 6   # max edge tiles per PSUM compute group (PSUM: 8 banks budget)
MAXM = 7   # max one-hot masks per compute group (7*256B < 1 PSUM bank)
NCH = 7    # AllGather chunks per kv table (NP must divide by NCH*128)


def _blockdiag(a):  # a: [H, D, D] -> [H*D, H*D]
    H, D, _ = a.shape
    out = np.zeros((H * D, H * D), np.float32)
    for h in range(H):
        out[h * D:(h + 1) * D, h * D:(h + 1) * D] = a[h]
    return out


def host_prep(inputs, cfg):
    N, E, HID, H, D, L, NC = (cfg[k] for k in ("N", "E", "HID", "H", "D", "L", "NC"))
    NSH = N // NC
    W = (NSH + 127) // 128
    NP = W * 128

    ip = {k: np.asarray(v) for k, v in inputs.items()}
    rel_st = [0, 1]
    rel_dt = [1, 0]
    edges = [ip["edge_ui"], ip["edge_iu"]]
    nch = NCH if NP % (NCH * 128) == 0 else (2 if NP % 256 == 0 else 1)
    CHR = NP // nch  # rows per AllGather chunk (chunk-major kvfull layout)

    # ---- effective weights ----
    scale = 1.0 / np.sqrt(D)
    W3 = np.zeros((L, 2, HID, 3 * HID), np.float32)  # (l, r): [Wk_eff|Wv_eff|Wq_eff(t=r)]
    b3 = np.zeros((L, 2, 3 * HID), np.float32)
    for l in range(L):
        for r in range(2):
            st, dt = rel_st[r], rel_dt[r]
            BDa = _blockdiag(ip["a_rel"][l, r])
            BDm = _blockdiag(ip["m_rel"][l, r])
            W3[l, r, :, 0:HID] = ip["Wk"][l, st] @ BDa
            b3[l, r, 0:HID] = ip["bk"][l, st] @ BDa
            W3[l, r, :, HID:2 * HID] = ip["Wv"][l, st] @ BDm
            b3[l, r, HID:2 * HID] = ip["bv"][l, st] @ BDm
        for t in range(2):
            r_of = 1 - t  # relation whose dst type is t
            pscale = np.repeat(ip["p_rel"][l, r_of] * scale, D)
            W3[l, t, :, 2 * HID:3 * HID] = ip["Wq"][l, t] * pscale[None, :]
            b3[l, t, 2 * HID:3 * HID] = ip["bq"][l, t] * pscale
    beta = 1.0 / (1.0 + np.exp(-ip["skip"]))  # [L, T]

    # ---- edge schedules (identical across cores) ----
    # Windows are packed into groups of up to KGRP consecutive windows that
    # share tiles (dense packing; boundary tiles carry one one-hot mask per
    # window they touch). Pool-engine gather count ~= tiles, so packing cuts
    # the dominant per-tile SWDGE cost by ~11%.
    KGRP = 3
    grpK = []
    wrem = W
    while wrem >= KGRP:
        grpK.append(KGRP)
        wrem -= KGRP
    if wrem:
        grpK.append(wrem)
    win2g = []
    for g, k in enumerate(grpK):
        win2g += [g] * k

    def prep_rel(e):
        src, dst = e[0].astype(np.int64), e[1].astype(np.int64)
        i_loc = src % NSH
        gsrc = (src // NSH) * NP + i_loc  # core-major (single-shot AllGather)
        per_core = []
        counts = np.zeros((NC, W), np.int64)
        for c in range(NC):
            sel = (dst // NSH) == c
            s_c = gsrc[sel]
            dl_c = dst[sel] - c * NSH
            order = np.argsort(dl_c, kind="stable")
            s_c, dl_c = s_c[order], dl_c[order]
            counts[c] = np.bincount(dl_c // 128, minlength=W)
            per_core.append((s_c, dl_c))
        wstart = np.concatenate([[0], np.cumsum(counts, axis=1)[:, :-1].T]).T \
            if False else np.concatenate(
                [np.zeros((NC, 1), np.int64), np.cumsum(counts, axis=1)[:, :-1]],
                axis=1)
        groups = []   # per group: dict(nt, tmasks=[(ti,wi)...], wb={wi:(lo,hi)})
        NT = 0
        wlo = 0
        for k in grpK:
            cnt_g = counts[:, wlo:wlo + k].sum(axis=1)
            nt = max(1, int((int(cnt_g.max()) + 127) // 128))
            wb = {}
            for wi in range(k):
                w = wlo + wi
                los, his = [], []
                for c in range(NC):
                    pre = int(counts[c, wlo:w].sum())
                    n = int(counts[c, w])
                    if n:
                        los.append(pre // 128)
                        his.append((pre + n - 1) // 128)
                lo = min(los) if los else 0
                hi = max(his) if his else 0
                wb[wi] = (lo, hi)
            tmasks = []
            for t in range(nt):
                for wi in range(k):
                    if wb[wi][0] <= t <= wb[wi][1]:
                        tmasks.append((t, wi))
            groups.append(dict(k=k, w0=wlo, nt=nt, tmasks=tmasks, wb=wb,
                               tbase=NT))
            NT += nt
            wlo += k
        NM = sum(len(g["tmasks"]) for g in groups)
        # per-core data: gather idx per tile, dloc per (tile, window-mask)
        idx_src = np.zeros((NC, NT * 128), np.int32)
        dloc = np.full((NC, NM * 128), 128.0, np.float32)  # 128 => miss
        for c in range(NC):
            s_c, dl_c = per_core[c]
            for g in groups:
                wlo = g["w0"]
                a0 = int(wstart[c, wlo])
                n_g = int(counts[c, wlo:wlo + g["k"]].sum())
                sl0 = g["tbase"] * 128
                idx_src[c, sl0:sl0 + n_g] = s_c[a0:a0 + n_g]
            mcol = 0
            for g in groups:
                wlo = g["w0"]
                for (t, wi) in g["tmasks"]:
                    w = wlo + wi
                    a = int(wstart[c, w])
                    n = int(counts[c, w])
                    # slots of window w on this core within the group stream
                    s_in_g = int(wstart[c, w] - wstart[c, wlo])
                    lo_s = max(s_in_g, t * 128)
                    hi_s = min(s_in_g + n, (t + 1) * 128)
                    if hi_s > lo_s:
                        aa = a + (lo_s - s_in_g)
                        dloc[c, mcol * 128 + (lo_s - t * 128):
                             mcol * 128 + (hi_s - t * 128)] = \
                            (dl_c[aa:aa + (hi_s - lo_s)] % 128).astype(np.float32)
                    mcol += 1
        return groups, NT, NM, idx_src, dloc

    schedules = []
    meta = []
    for r in range(2):
        groups, NT, NM, idx_src, dloc = prep_rel(edges[r])
        schedules.append((groups, NT, NM))
        meta.append((idx_src, dloc))

    # ---- per-core input arrays ----
    NTtot = schedules[0][1] + schedules[1][1]
    NMtot = schedules[0][2] + schedules[1][2]
    xs = [ip["x_user"].astype(np.float32), ip["x_item"].astype(np.float32)]
    in_maps = []
    for c in range(NC):
        x_fm = np.zeros((2, HID, NP), BF)
        for t in range(2):
            x_fm[t, :, :NSH] = xs[t][c * NSH:(c + 1) * NSH].T.astype(BF)
        idx_cat = np.concatenate(
            [meta[0][0][c], meta[1][0][c]]).reshape(NTtot, 128).T
        dl = np.concatenate([meta[0][1][c], meta[1][1][c]])
        dloc_col = dl.reshape(NMtot, 128).T.astype(np.float32).copy()
        in_maps.append({
            "x_fm": x_fm,
            "idx_src": np.ascontiguousarray(idx_cat.astype(np.int32)),
            "dloc_col": np.ascontiguousarray(dloc_col),
        })

    # bias cols [128, NB] f32: 0,1 = b_in; 2.. = beta*bo (l,t)
    b_list = [ip["b_in"][0], ip["b_in"][1]]
    for l in range(L):
        for t in range(2):
            b_list.append(beta[l, t] * ip["bo"][l, t])
    Bcols = np.stack(b_list).astype(np.float32)

    bias_nz = [[bool(np.any(b3[l, r] != 0)) for r in range(2)] for l in range(L)]
    iota = np.tile(np.arange(128, dtype=np.float32), (128, 1))
    consts = {
        "bias_nz": bias_nz,
        "iota_row": np.tile(iota, (1, MAXM)).astype(BF),  # [128, MAXM*128]
        "ident": np.eye(128, dtype=np.float32).astype(BF),
        "W3": W3.reshape(L * 2, HID, 3 * HID).astype(BF),
        "Win": ip["W_in"].astype(np.float32).astype(BF),
        "Wo_bf": ip["Wo"].astype(np.float32).reshape(L * 2, HID, HID).astype(BF),
        "b3": b3.reshape(1, L * 2 * 3 * HID).astype(np.float32),
        "Bcols": Bcols,
        "ones1f": np.ones((1, 128), np.float32),
    }
    dims = dict(NSH=NSH, W=W, NP=NP, NTtot=NTtot, NMtot=NMtot,
                NCHR=nch, win2g=win2g, NG=len(grpK))
    return in_maps, consts, {}, schedules, dims, beta


def build_program(cfg, consts, bases, schedules, dims, beta, sim_gelu=False):
    N, E, HID, H, D, L, NC = (cfg[k] for k in ("N", "E", "HID", "H", "D", "L", "NC"))
    NSH, W, NP, NTtot = dims["NSH"], dims["W"], dims["NP"], dims["NTtot"]
    NMtot, win2g, NG = dims["NMtot"], dims["win2g"], dims["NG"]
    NPALL = NP * NC
    rel_dt = [1, 0]
    NB = consts["Bcols"].shape[0]
    CHD = 448 if NP % 448 == 0 else 128      # dense (psum) chunk width
    nch = dims["NCHR"]
    CHN = NP // nch                           # node h chunk == AG chunk rows
    assert NP % CHD == 0 and NP % CHN == 0 and CHN % 128 == 0

    nc = bacc.Bacc("TRN2", target_bir_lowering=False, debug=False, num_devices=NC)

    x_fm = nc.dram_tensor("x_fm", [2, HID, NP], bf16, kind="ExternalInput")
    idx_src = nc.dram_tensor("idx_src", [128, NTtot], i32, kind="ExternalInput")
    dloc_col_d = nc.dram_tensor("dloc_col", [128, NMtot], f32, kind="ExternalInput")
    it_row_d = nc.dram_tensor("iota_row", [128, MAXM * 128], bf16, kind="ExternalInput")
    ident_d = nc.dram_tensor("ident", [128, 128], bf16, kind="ExternalInput")
    W3_d = nc.dram_tensor("W3", [L * 2, HID, 3 * HID], bf16, kind="ExternalInput")
    Win_d = nc.dram_tensor("Win", [2, HID, HID], bf16, kind="ExternalInput")
    Wo_d = nc.dram_tensor("Wo_bf", [L * 2, HID, HID], bf16, kind="ExternalInput")
    b3_d = nc.dram_tensor("b3", [1, L * 2 * 3 * HID], f32, kind="ExternalInput")
    Bcols_d = nc.dram_tensor("Bcols", [NB, HID], f32, kind="ExternalInput")
    ones1f_d = nc.dram_tensor("ones1f", [1, 128], f32, kind="ExternalInput")
    out_d = nc.dram_tensor("out", [2, HID, NP], f32, kind="ExternalOutput")

    with tile.TileContext(nc) as tc:
        with tc.tile_pool(name="persist", bufs=1) as pp, \
             tc.tile_pool(name="dram", bufs=1, space="DRAM") as dp, \
             tc.tile_pool(name="wk_sb", bufs=3) as sb3, \
             tc.tile_pool(name="wk_sb2", bufs=2) as sb2, \
             tc.tile_pool(name="gath", bufs=8) as gpool, \
             tc.tile_pool(name="edge8", bufs=2) as sb8, \
             tc.tile_pool(name="ps_edge", bufs=2, space="PSUM") as ps_e, \
             tc.tile_pool(name="ps_dense", bufs=2, space="PSUM") as ps_d:

            # --- persistent SBUF ---
            it_row = pp.tile([128, MAXM, 128], bf16)
            nc.sync.dma_start(it_row[:], it_row_d[:].rearrange(
                "p (g d) -> p g d", g=MAXM))
            ident = pp.tile([128, 128], bf16)
            nc.sync.dma_start(ident[:], ident_d[:])
            onesf = pp.tile([1, 128], f32)
            nc.sync.dma_start(onesf[:], ones1f_d[:])
            idxs = pp.tile([128, NTtot], i32)
            nc.sync.dma_start(idxs[:], idx_src[:])
            dloc_col = pp.tile([128, NMtot], f32)
            nc.sync.dma_start(dloc_col[:], dloc_col_d[:])
            w3sb = pp.tile([128, L * 2, 3 * HID], bf16)
            nc.sync.dma_start(w3sb[:], W3_d[:].rearrange("k p d -> p k d"))
            winsb = pp.tile([128, 2, HID], bf16)
            nc.sync.dma_start(winsb[:], Win_d[:].rearrange("k p d -> p k d"))
            wosb = pp.tile([128, L * 2, HID], bf16)
            nc.sync.dma_start(wosb[:], Wo_d[:].rearrange("k p d -> p k d"))
            b3sb = pp.tile([1, L * 2 * 3 * HID], f32)
            nc.sync.dma_start(b3sb[:], b3_d[:])
            bcols = pp.tile([128, NB], f32)
            nc.sync.dma_start(bcols[:], Bcols_d[:].rearrange("k d -> d k"))

            q_sb = [pp.tile([128, W, 128], bf16, name=f"q_sb{t}") for t in range(2)]
            g_fm = [pp.tile([128, NP], bf16, name=f"g_fm{t}") for t in range(2)]

            hA = [dp.tile([128, NP], bf16, name=f"hA{t}") for t in range(2)]
            hB = [dp.tile([128, NP], bf16, name=f"hB{t}") for t in range(2)]
            kvloc = [dp.tile([NP, 256], fp8, name=f"kvloc{r}") for r in range(2)]
            kvfull = [[dp.tile([NPALL, 256], fp8, name=f"kvfull{l}{r}",
                        addr_space="Shared")
                       for r in range(2)] for l in range(L)]
            rg = [list(range(NC))]

            def b3row(l, r, lo, hi):  # bias row slice [1, hi-lo]
                base = (l * 2 + r) * 3 * HID
                return b3sb[:, base + lo:base + hi]

            def ag_full(l, r):
                # One-shot AllGather into the pair-Shared kvfull (Shared
                # outputs allow a single writer instruction only). Core c's
                # kvloc lands at rows [c*NP, (c+1)*NP) (core-major gsrc).
                nc.gpsimd.collective_compute(
                    "AllGather", ALU.bypass, replica_groups=rg,
                    ins=[kvloc[r][:, :]],
                    outs=[kvfull[l][r][:, :]])

            # dense projection pass over the node shard, writing kv and/or q.
            def node_pass(l, r, h_src, do_kv, do_q):
                lo = 0 if do_kv else 2 * HID
                hi = 3 * HID if do_q else 2 * HID
                ncols = hi - lo
                for jc in range(NP // CHN):
                    hch = sb3.tile([128, CHN], bf16, tag="hch")
                    nc.sync.dma_start(hch[:], h_src[:, jc * CHN:(jc + 1) * CHN])
                    for k in range(CHN // 128):
                        w = jc * (CHN // 128) + k
                        ps = ps_d.tile([128, 3 * HID], f32, tag="dense")
                        bias_nz = consts["bias_nz"][l][r]
                        nc.tensor.matmul(
                            out=ps[:, :ncols], lhsT=hch[:, k * 128:(k + 1) * 128],
                            rhs=w3sb[:, l * 2 + r, lo:hi], start=True,
                            stop=not bias_nz)
                        if bias_nz:
                            nc.tensor.matmul(
                                out=ps[:, :ncols], lhsT=onesf[:],
                                rhs=b3row(l, r, lo, hi), start=False, stop=True)
                        if do_kv:
                            kvt = sb3.tile([128, 256], fp8, tag="kvt")
                            nc.scalar.activation(kvt[:], ps[:, 0:256], AF.Copy)
                            nc.sync.dma_start(
                                kvloc[r][w * 128:(w + 1) * 128, :], kvt[:])
                        if do_q:
                            nc.vector.tensor_copy(
                                q_sb[r][:, w, :], ps[:, ncols - HID:ncols])


            def flush_window(dt, w, pw):
                # normalize window agg, transpose into g_fm
                zrw = sb8.tile([128, 4], f32, tag="zrw")
                nc.vector.tensor_scalar(out=zrw[:], in0=pw[:, 128:132],
                                        scalar1=1e-16, scalar2=None,
                                        op0=ALU.add)
                nc.vector.reciprocal(zrw[:], zrw[:])
                gt = sb8.tile([128, 128], bf16, tag="gt")
                nc.vector.tensor_tensor(
                    out=gt[:].rearrange("p (h d) -> p h d", h=H),
                    in0=pw[:, 0:128].rearrange("p (h d) -> p h d", h=H),
                    in1=zrw[:].to_broadcast([128, H, D]),
                    op=ALU.mult)
                psgt = ps_e.tile([128, 128], bf16, tag="st")
                nc.tensor.transpose(out=psgt[:], in_=gt[:], identity=ident[:])
                nc.scalar.activation(g_fm[dt][:, w * 128:(w + 1) * 128],
                                     psgt[:], AF.Copy)

            def edge_phase(l, r, tbase, mbase, hooks=None):
                # hooks: ordered [(min_group, closure)] spliced between
                # window-groups, fired strictly in list order (at most 4 per
                # group) once min_group's flush has been emitted. min_group
                # must be >= the group whose flush produces the closure's
                # input (dependency-order within the list).
                pending = list(hooks or [])
                groups, NT, NM = schedules[r]
                dt = rel_dt[r]
                for gidx, gd in enumerate(groups):
                    K, w0, nt, tmasks, wb = (gd["k"], gd["w0"], gd["nt"],
                                             gd["tmasks"], gd["wb"])
                    tb0 = tbase + gd["tbase"]
                    # per-window accumulators (zero-region = one full bank:
                    # interleaved groups must live in separate banks; only
                    # adjacent windows overlap, ring of 2 suffices)
                    pswin = {}
                    flushed = [False] * K
                    # split group tiles into compute sub-groups bounded by
                    # GMAX tiles and MAXM one-hot masks
                    nmask_of = [0] * nt
                    for (t, wi) in tmasks:
                        nmask_of[t] += 1
                    ta = 0
                    moff = 0  # mask offset within group
                    while ta < nt:
                        tb_ = ta
                        nm = 0
                        while (tb_ < nt and tb_ - ta < GMAX
                               and nm + nmask_of[tb_] <= MAXM):
                            nm += nmask_of[tb_]
                            tb_ += 1
                        Gt = tb_ - ta
                        sg_masks = [mk for mk in tmasks if ta <= mk[0] < tb_]
                        NMsg = len(sg_masks)
                        mb = mbase + moff
                        ts = tb0 + ta
                        # gather the sub-group's kv rows (one DMA per
                        # 128-edge tile; batched offsets mislower on HW)
                        kvg = gpool.tile([128, GMAX, 256], fp8, tag="g")
                        for i in range(Gt):
                            tt = ts + i
                            nc.gpsimd.indirect_dma_start(
                                out=kvg[:, i, :], out_offset=None,
                                in_=kvfull[l][r][:],
                                in_offset=bass.IndirectOffsetOnAxis(
                                    ap=idxs[:, tt:tt + 1], axis=0))
                        # one-hot masks for every (tile, window) pair
                        S2 = sb8.tile([128, MAXM, 128], bf16, tag="S")
                        nc.vector.tensor_tensor(
                            out=S2[:, :NMsg, :], in0=it_row[:, :NMsg, :],
                            in1=dloc_col[:, mb:mb + NMsg].to_broadcast(
                                [128, NMsg, 128]),
                            op=ALU.is_equal)
                        psst = ps_e.tile([128, MAXM, 128], bf16, tag="st")
                        for m in range(NMsg):
                            nc.tensor.transpose(out=psst[:, m, :],
                                                in_=S2[:, m, :],
                                                identity=ident[:])
                        St2 = sb8.tile([128, MAXM, 128], bf16, tag="St")
                        nc.scalar.activation(St2[:, :NMsg, :], psst[:, :NMsg, :],
                                             AF.Copy)
                        # q[dst] per slot: accumulate this tile's masks
                        psqe = ps_e.tile([128, GMAX, 128], f32, tag="qe",
                                         bufs=1)
                        for i in range(Gt):
                            mks = [m for m, mk in enumerate(sg_masks)
                                   if mk[0] == ta + i]
                            for j, m in enumerate(mks):
                                nc.tensor.matmul(
                                    out=psqe[:, i, :], lhsT=St2[:, m, :],
                                    rhs=q_sb[dt][:, w0 + sg_masks[m][1], :],
                                    start=(j == 0), stop=(j == len(mks) - 1))
                        qk = sb8.tile([128, GMAX, 128], bf16, tag="qk")
                        nc.vector.tensor_tensor(
                            out=qk[:, :Gt, :], in0=psqe[:, :Gt, :],
                            in1=kvg[:, 0:Gt, 0:128], op=ALU.mult)
                        lg = sb8.tile([128, GMAX, 4], f32, tag="lg")
                        nc.vector.tensor_reduce(
                            out=lg[:, :Gt, :],
                            in_=qk[:, :Gt, :].rearrange(
                                "p g (h d) -> p (g h) d", h=H),
                            axis=mybir.AxisListType.X, op=ALU.add)
                        pay = sb8.tile([128, GMAX, 132], bf16, tag="pay")
                        nc.scalar.activation(pay[:, :Gt, 128:132], lg[:, :Gt, :],
                                             AF.Exp)
                        nc.vector.tensor_tensor(
                            out=pay[:, :Gt, 0:128].rearrange(
                                "p g (h d) -> p g h d", h=H),
                            in0=kvg[:, 0:Gt, 128:256].rearrange(
                                "p g (h d) -> p g h d", h=H),
                            in1=pay[:, :Gt, 128:132].to_broadcast(
                                [128, Gt, H, D]),
                            op=ALU.mult)
                        # per-window segment accumulation (one matmul per mask)
                        for m, (t, wi) in enumerate(sg_masks):
                            if wi not in pswin:
                                pswin[wi] = ps_e.tile([128, 132], f32,
                                                      tag="win", name="pswin")
                            nc.tensor.matmul(
                                out=pswin[wi][:], lhsT=S2[:, m, :],
                                rhs=pay[:, t - ta, :],
                                start=(t == wb[wi][0]), stop=(t == wb[wi][1]))
                        ta = tb_
                        moff += NMsg
                        # flush windows whose last tile has completed
                        for wi in range(K):
                            if not flushed[wi] and wb[wi][1] < ta:
                                flush_window(dt, w0 + wi, pswin.pop(wi))
                                flushed[wi] = True
                    mbase += len(tmasks)
                    for wi in range(K):
                        if not flushed[wi]:
                            flush_window(dt, w0 + wi, pswin.pop(wi))
                            flushed[wi] = True
                    fired = 0
                    while pending and pending[0][0] <= gidx and fired < 4:
                        pending.pop(0)[1]()
                        fired += 1
                for _, fn in pending:
                    fn()

            def bulk_gelu(t, lo, hi):
                if not sim_gelu:
                    nc.scalar.activation(g_fm[t][:, lo:hi], g_fm[t][:, lo:hi],
                                         AF.Gelu)
                else:
                    tmp = sb2.tile([128, NP], f32, tag="sgl")
                    g = g_fm[t][:, lo:hi]
                    tm = tmp[:, lo:hi]
                    nc.vector.tensor_tensor(out=tm, in0=g, in1=g, op=ALU.mult)
                    nc.vector.tensor_scalar(out=tm, in0=tm, scalar1=0.044715,
                                            scalar2=1.0, op0=ALU.mult, op1=ALU.add)
                    nc.vector.tensor_tensor(out=tm, in0=tm, in1=g, op=ALU.mult)
                    nc.scalar.activation(tm, tm, AF.Tanh, scale=0.7978845608028654)
                    nc.vector.tensor_scalar(out=tm, in0=tm, scalar1=1.0, scalar2=0.5,
                                            op0=ALU.add, op1=ALU.mult)
                    nc.vector.tensor_tensor(out=g, in0=tm, in1=g, op=ALU.mult)

            def out_chunk(l, t, h_src, dst, last, j):
                bb = 2 + l * 2 + t
                coef = float((1.0 - beta[l, t]) + (1.0 if l > 0 else 0.0))
                sl = slice(j * CHD, (j + 1) * CHD)
                bulk_gelu(t, j * CHD, (j + 1) * CHD)
                ps = ps_d.tile([128, CHD], f32, tag="dense")
                nc.tensor.matmul(out=ps[:], lhsT=wosb[:, l * 2 + t, :],
                                 rhs=g_fm[t][:, sl], start=True, stop=True)
                a1 = sb2.tile([128, CHD], f32, tag="a1")
                nc.vector.tensor_scalar(
                    out=a1[:], in0=ps[:], scalar1=float(beta[l, t]),
                    scalar2=bcols[:, bb:bb + 1], op0=ALU.mult, op1=ALU.add)
                hch = sb2.tile([128, CHD], bf16, tag="hcho")
                nc.sync.dma_start(hch[:], h_src[:, sl])
                hn = sb2.tile([128, CHD], f32 if last else bf16, tag="hn")
                nc.vector.scalar_tensor_tensor(
                    out=hn[:], in0=hch[:], scalar=coef, in1=a1[:],
                    op0=ALU.mult, op1=ALU.add)
                nc.sync.dma_start(dst[:, sl], hn[:])

            def out_chunks(l, t, h_src, dst, last):
                return [(lambda j=j: out_chunk(l, t, h_src, dst, last, j))
                        for j in range(NP // CHD)]

            def node_pass_chunk(l, r, h_src, do_kv, do_q, jc, ag=None):
                # one CHN-wide chunk of node_pass (see node_pass)
                lo = 0 if do_kv else 2 * HID
                hi = 3 * HID if do_q else 2 * HID
                ncols = hi - lo
                hch = sb3.tile([128, CHN], bf16, tag="hch")
                nc.sync.dma_start(hch[:], h_src[:, jc * CHN:(jc + 1) * CHN])
                for k in range(CHN // 128):
                    w = jc * (CHN // 128) + k
                    ps = ps_d.tile([128, 3 * HID], f32, tag="dense")
                    bias_nz = consts["bias_nz"][l][r]
                    nc.tensor.matmul(
                        out=ps[:, :ncols], lhsT=hch[:, k * 128:(k + 1) * 128],
                        rhs=w3sb[:, l * 2 + r, lo:hi], start=True,
                        stop=not bias_nz)
                    if bias_nz:
                        nc.tensor.matmul(
                            out=ps[:, :ncols], lhsT=onesf[:],
                            rhs=b3row(l, r, lo, hi), start=False, stop=True)
                    if do_kv:
                        kvt = sb3.tile([128, 256], fp8, tag="kvt")
                        nc.scalar.activation(kvt[:], ps[:, 0:256], AF.Copy)
                        nc.sync.dma_start(
                            kvloc[r][w * 128:(w + 1) * 128, :], kvt[:])
                    if do_q:
                        nc.vector.tensor_copy(
                            q_sb[r][:, w, :], ps[:, ncols - HID:ncols])
                if ag is not None and jc == NP // CHN - 1:
                    ag_full(ag[0], ag[1])

            def node_pass_chunks(l, r, h_src, do_kv, do_q, ag=None):
                return [(lambda jc=jc: node_pass_chunk(l, r, h_src, do_kv,
                                                       do_q, jc, ag))
                        for jc in range(NP // CHN)]

            def input_proj(t, dst):
                for j in range(NP // CHD):
                    sl = slice(j * CHD, (j + 1) * CHD)
                    xt = sb2.tile([128, CHD], bf16, tag="xt")
                    nc.sync.dma_start(xt[:], x_fm[t, :, sl])
                    ps = ps_d.tile([128, CHD], f32, tag="dense")
                    nc.tensor.matmul(out=ps[:], lhsT=winsb[:, t, :], rhs=xt[:],
                                     start=True, stop=True)
                    ht = sb2.tile([128, CHD], bf16, tag="ht")
                    nc.scalar.activation(ht[:], ps[:], AF.Relu,
                                         bias=bcols[:, t:t + 1], scale=1.0)
                    nc.sync.dma_start(dst[:, sl], ht[:])

            # ---------------- schedule ----------------
            # All dense work (out-projection chunks, next-layer node-pass
            # chunks, AllGather chunks) is spliced INTO the edge phases as
            # dependency-gated hooks, so no engine queue ever drains behind
            # a dense block and the Pool gather stream never stops:
            #   edge(l, rF)  hooks: AG(l, rS) chunks  [inputs ready at start]
            #   edge(l, rS)  hooks: out(l, tF) packed early; kv node-pass +
            #                AG for (l+1, rF2); out(l, tS) gated on this
            #                phase's window flushes; q node-pass for (l+1,
            #                rS2) gated on out(l, tS) chunks.
            #   edge(l+1, rF2) hooks: kv node-pass + AG for (l+1, rS2) and
            #                q node-pass for (l+1, rF2)  [inputs ready]
            rorder = [[0, 1] if l % 2 == 0 else [1, 0] for l in range(L)]
            tb = [0, schedules[0][1]]   # tile base per relation
            mb_ = [0, schedules[0][2]]  # mask-column base per relation
            NNC = NP // CHN             # node-pass chunks

            def g_of(w):  # window index -> hook gating group index
                return win2g[min(w, W - 1)]

            h_cur = hA
            rF, rS = rorder[0]
            input_proj(rF, hA[rF])
            node_pass(0, rF, hA[rF], do_kv=True, do_q=False)
            ag_full(0, rF)
            input_proj(rS, hA[rS])
            node_pass(0, rS, hA[rS], do_kv=True, do_q=True)
            node_pass(0, rF, hA[rF], do_kv=False, do_q=True)
            hooksA = [(g_of(55), lambda: ag_full(0, rS))]

            for l in range(L):
                rF, rS = rorder[l]
                last = l == L - 1
                l2 = l + 1
                h_nxt = hB if l == 0 else None
                edge_phase(l, rF, tb[rF], mb_[rF], hooks=hooksA)

                tF_out = rel_dt[rF]
                tS_out = rel_dt[rS]
                dstF = (h_nxt[tF_out] if not last else out_d[tF_out])
                dstS = (h_nxt[tS_out] if not last else out_d[tS_out])
                hooks2 = [(g_of(2 + j), fn) for j, fn in
                          enumerate(out_chunks(l, tF_out, h_cur[tF_out],
                                               dstF, last))]
                # out(l, tS) chunk j needs this phase's window
                # floor(((j+1)*CHD-1)/128) flushed
                hooks2 += [(g_of((CHD * (j + 1) - 1) // 128), fn) for j, fn in
                           enumerate(out_chunks(l, tS_out, h_cur[tS_out],
                                                dstS, last))]
                hooks2.sort(key=lambda e: e[0])
                edge_phase(l, rS, tb[rS], mb_[rS], hooks=hooks2)

                if not last:
                    rF2, rS2 = rorder[l2]
                    for fn in node_pass_chunks(l2, rF2, h_nxt[rF2], True,
                                               False, ag=(l2, rF2)):
                        fn()
                    for fn in node_pass_chunks(l2, rS2, h_nxt[rS2], False,
                                               True):
                        fn()

                if not last:
                    hooksA = []
                    npkv = node_pass_chunks(l2, rS2, h_nxt[rS2], True, False,
                                            ag=(l2, rS2))
                    npq = node_pass_chunks(l2, rF2, h_nxt[rF2], False, True)
                    for jc in range(NNC):
                        hooksA.append((g_of(12 + 3 * jc), npkv[jc]))
                        hooksA.append((g_of(2 + 2 * jc), npq[jc]))
                    hooksA.sort(key=lambda e: e[0])
                    h_cur = hB

    nc.finalize()
    return nc


def run(inputs, cfg=None, trace=False, trace_cores=None, sim=False):
    cfg = cfg or FULL_CFG
    NC = cfg["NC"]
    core_maps, consts, bases, schedules, dims, beta = host_prep(inputs, cfg)
    nc = build_program(cfg, consts, bases, schedules, dims, beta, sim_gelu=sim)
    in_maps = []
    for c in range(NC):
        m = dict(core_maps[c])
        for k in ("iota_row", "ident", "W3", "Win", "Wo_bf", "b3", "Bcols",
                  "ones1f"):
            m[k] = consts[k]
        in_maps.append(m)
    if sim:
        from concourse.bass_interp import MultiCoreSim

        msim = MultiCoreSim(nc, num_cores=NC, trace=False,
                            require_finite=False, require_nnan=False)
        cores = [msim.cores[c] for c in range(NC)]
        for c in range(NC):
            for name, arr in in_maps[c].items():
                cores[c].tensor(name)[:] = arr
        msim.simulate(check_with_hw=False)

        class R:
            exec_time_ns = None
            results = [{"out": np.asarray(cores[c].tensor("out"))}
                       for c in range(NC)]
        res = R()
    else:
        res = run_bass_kernel_spmd(nc, in_maps, core_ids=list(range(NC)),
                                   trace=trace, trace_cores=trace_cores)
    NSH, NP = dims["NSH"], dims["NP"]
    out = np.empty((2, cfg["N"], cfg["HID"]), np.float32)
    for c in range(NC):
        o = res.results[c]["out"]
        for t in range(2):
            out[t, c * NSH:(c + 1) * NSH] = o[t, :, :NSH].T
    return out, res


def kernel(**inputs):
    out, _ = run(inputs, FULL_CFG, trace=False)
    return out

